# revision 1
# baseline (speedup 1.0000x reference)
"""Self-contained Trainium2 Bass kernel for nn_Attention_59253368816224.

GQA attention block: per-head RMSNorm on Q/K, RoPE, causal softmax
attention, o_proj.  B=2, S=2048, H=2048, 16 Q heads / 4 KV heads,
head_dim=128.

Sharding: 8 cores = 2 batches x 4 KV groups.  Core c -> (b=c//4, g=c%4)
owns 4 Q heads + 1 KV head.  o_proj is row-parallel: each core emits a
partial [S, H] output contracted over its 512 hidden dims; the host sums
the 4 partials per batch in fp32.

Device pipeline (all matmuls bf16 with fp32 PSUM accumulation):
  A) W-stationary QKV projection from host-pre-transposed xT, producing
     transposed qT/kT/vT [d, s]; per-column sum-of-squares via selector
     matmuls -> RMSNorm scales rsqrt(mean+eps) = exp(-0.5*ln(.)) on ACT.
  B) RoPE in the transposed domain: rot(q) via a constant 128x128
     permutation matmul; combine with w-folded cosT/sinT tables; apply
     the norm scale broadcast across partitions via a DMA row-replicate.
  C) v transposed back to natural [s, d] with a DMA transpose.
  D) Flash-style causal attention on transposed scores [j, i]:
     exp on ACT (scale 1/sqrt(128)), triangle masks on GpSimd, attn@v and
     an all-ones rowsum matmul accumulate in PSUM; normalize via
     exp(-ln(rowsum)) during the PSUM->SBUF move.
  E) o_proj from oT tiles against Wo rows, partial output to DRAM bf16.
"""

import os
import sys
import numpy as np
import ml_dtypes

BF16 = ml_dtypes.bfloat16

B = 2
S = 2048
H = 2048
NQH = 16          # total q heads
NKV = 4           # total kv heads
HD = 128          # head dim
GQ = 4            # q heads per core (per kv group)
KT = H // 128     # 16 k-tiles over hidden
ST = S // 128     # 16 s-tiles
RMS_EPS = 1.1920928955078125e-07
INV_SQRT_HD = 1.0 / float(np.sqrt(HD))

_PROGRAM = None


def _build_program():
    import concourse.bacc as bacc
    import concourse.tile as tile
    from concourse import mybir
    from contextlib import ExitStack

    bf = mybir.dt.bfloat16
    f32 = mybir.dt.float32

    nc = bacc.Bacc("TRN2", target_bir_lowering=False, debug=False, num_devices=8)

    # ---- DRAM I/O (per-core values supplied via in_maps) ----
    xt_d = nc.dram_tensor("xt", (H, S), bf, kind="ExternalInput")
    wq_d = nc.dram_tensor("wq", (H, GQ * HD), bf, kind="ExternalInput")
    wk_d = nc.dram_tensor("wk", (H, HD), bf, kind="ExternalInput")
    wv_d = nc.dram_tensor("wv", (H, HD), bf, kind="ExternalInput")
    wo_d = nc.dram_tensor("wo", (GQ * HD, H), bf, kind="ExternalInput")
    cosq_d = nc.dram_tensor("cosq", (HD, S), bf, kind="ExternalInput")
    sinq_d = nc.dram_tensor("sinq", (HD, S), bf, kind="ExternalInput")
    cosk_d = nc.dram_tensor("cosk", (HD, S), bf, kind="ExternalInput")
    sink_d = nc.dram_tensor("sink", (HD, S), bf, kind="ExternalInput")
    rmat_d = nc.dram_tensor("rmat", (128, 128), bf, kind="ExternalInput")
    ones_d = nc.dram_tensor("ones", (128, 128), bf, kind="ExternalInput")
    mask_d = nc.dram_tensor("mask", (4, 128, 512), bf, kind="ExternalInput")
    out_d = nc.dram_tensor("out", (S, H), bf, kind="ExternalOutput")
    # internal scratch for the scale-row broadcast and the v transpose
    vt_scratch = nc.dram_tensor("vt_scratch", (HD, S), bf)

    Exp = mybir.ActivationFunctionType.Exp
    Ln = mybir.ActivationFunctionType.Ln
    Square = mybir.ActivationFunctionType.Square

    with tile.TileContext(nc) as tc:
        with ExitStack() as ctx:
            consts = ctx.enter_context(tc.tile_pool(name="consts", bufs=1))
            persist = ctx.enter_context(tc.tile_pool(name="persist", bufs=1))

            # ---- persistent intermediates ----
            qkvbf = persist.tile([128, 5, S], bf)      # raw transposed q(4)/k
            vt = persist.tile([128, S], bf)            # raw transposed v
            qfin = persist.tile([128, GQ, S], bf)      # roped+normed qT
            kfin = persist.tile([128, S], bf)          # roped+normed kT
            v3 = persist.tile([128, ST, HD], bf)       # v natural [jt][j][d]
            otsb = persist.tile([128, GQ, S], bf)      # oT per head

            # ---- constant tiles ----
            cosq = consts.tile([128, S], bf)
            sinq = consts.tile([128, S], bf)
            cosk = consts.tile([128, S], bf)
            sink = consts.tile([128, S], bf)
            rmat = consts.tile([128, 128], bf)
            onesm = consts.tile([128, 128], bf)
            masks = consts.tile([128, 4, 512], bf)
            eps128 = consts.tile([128, 1], mybir.dt.float32)
            nc.vector.memset(eps128[:], RMS_EPS)

            # ============ Phase A: QKV projection (+ per-chunk scales) ====
            CHUNK_ORDER = [4, 5, 0, 1, 2, 3]
            with tc.tile_pool(name="proj_in", bufs=1) as proj_in, \
                 tc.tile_pool(name="sqp", bufs=2) as sqp, \
                 tc.tile_pool(name="scb", bufs=2) as scbp, \
                 tc.tile_pool(name="ropet", bufs=2) as ropet, \
                 tc.tile_pool(name="qkv_ps", bufs=2, space="PSUM") as qkv_ps, \
                 tc.tile_pool(name="ss_ps", bufs=1, space="PSUM") as ss_ps, \
                 tc.tile_pool(name="rot_ps", bufs=2, space="PSUM") as rot_psp:
                xts = [proj_in.tile([128, S], bf, tag=f"xt{k}", name=f"xt{k}") for k in range(KT)]
                wqs = [proj_in.tile([128, GQ * HD], bf, tag=f"wq{k}", name=f"wqs{k}") for k in range(KT)]
                wks = [proj_in.tile([128, HD], bf, tag=f"wk{k}", name=f"wks{k}") for k in range(KT)]
                wvs = [proj_in.tile([128, HD], bf, tag=f"wv{k}", name=f"wvs{k}") for k in range(KT)]
                # k-major, split issue: early k-tiles land first
                for k in range(KT):
                    for q2 in range(2):
                        nc.sync.dma_start(
                            out=xts[k][:, q2 * 1024:(q2 + 1) * 1024],
                            in_=xt_d[k * 128:(k + 1) * 128, q2 * 1024:(q2 + 1) * 1024],
                        )
                    nc.sync.dma_start(out=wks[k][:], in_=wk_d[k * 128:(k + 1) * 128, :])
                for k in range(KT):
                    nc.sync.dma_start(out=wvs[k][:], in_=wv_d[k * 128:(k + 1) * 128, :])
                for k in range(KT):
                    nc.sync.dma_start(out=wqs[k][:], in_=wq_d[k * 128:(k + 1) * 128, :])
                for hh in range(2):
                    sl = slice(hh * 1024, (hh + 1) * 1024)
                    nc.sync.dma_start(out=cosk[:, sl], in_=cosk_d[:, sl])
                    nc.sync.dma_start(out=sink[:, sl], in_=sink_d[:, sl])
                nc.sync.dma_start(out=rmat[:], in_=rmat_d[:])
                for hh in range(2):
                    sl = slice(hh * 1024, (hh + 1) * 1024)
                    nc.sync.dma_start(out=cosq[:, sl], in_=cosq_d[:, sl])
                    nc.sync.dma_start(out=sinq[:, sl], in_=sinq_d[:, sl])
                nc.sync.dma_start(out=onesm[:], in_=ones_d[:])
                for tt in range(4):
                    nc.sync.dma_start(
                        out=masks[:, tt, :], in_=mask_d[tt, :, :]
                    )

                scbs = {}
                for c in CHUNK_ORDER:
                    for half in range(2):
                        h0 = half * 1024
                        ps = qkv_ps.tile([128, 1024], mybir.dt.float32)
                        for k in range(KT):
                            if c < 4:
                                lhsT = wqs[k][:, c * 128:(c + 1) * 128]
                            elif c == 4:
                                lhsT = wks[k][:]
                            else:
                                lhsT = wvs[k][:]
                            for nn in range(2):
                                nc.tensor.matmul(
                                    ps[:, nn * 512:(nn + 1) * 512],
                                    lhsT,
                                    xts[k][:, h0 + nn * 512:h0 + (nn + 1) * 512],
                                    start=(k == 0),
                                    stop=(k == KT - 1),
                                )
                        dst = (
                            qkvbf[:, c, h0:h0 + 1024]
                            if c < 5
                            else vt[:, h0:h0 + 1024]
                        )
                        if (c + half) % 2 == 0:
                            nc.scalar.copy(dst, ps[:])
                        else:
                            nc.vector.tensor_copy(dst, ps[:])
                        if c < 5:
                            sq = sqp.tile([128, 1024], bf)
                            nc.gpsimd.tensor_mul(sq[:], dst, dst)
                            sst = ss_ps.tile([128, 1024], mybir.dt.float32)
                            for nn in range(2):
                                nc.tensor.matmul(
                                    sst[:, nn * 512:(nn + 1) * 512],
                                    onesm[:],
                                    sq[:, nn * 512:(nn + 1) * 512],
                                    start=True,
                                    stop=True,
                                )
                            # scale = rsqrt(mean+eps), broadcast on all rows
                            scb = scbp.tile([128, 1024], mybir.dt.float32)
                            scbs[(c, half)] = scb
                            nc.scalar.activation(
                                scb[:], sst[:], Ln, bias=eps128[:], scale=1.0 / HD
                            )
                            nc.scalar.activation(scb[:], scb[:], Exp, scale=-0.5)
                        elif half == 1:
                            nc.sync.dma_start(out=vt_scratch[:], in_=vt[:])
                            nc.sync.dma_start_transpose(out=v3[:], in_=vt_scratch[:])
                    if c < 5:
                        cosx = cosq if c < 4 else cosk
                        sinx = sinq if c < 4 else sink
                        for q4 in range(4):
                            o0 = q4 * 512
                            scb = scbs[(c, o0 // 1024)]
                            src_ap = qkvbf[:, c, o0:o0 + 512]
                            rot = rot_psp.tile([128, 512], mybir.dt.float32)
                            nc.tensor.matmul(
                                rot[:], rmat[:], src_ap, start=True, stop=True
                            )
                            a = ropet.tile([128, 512], bf, tag="a")
                            bb = ropet.tile([128, 512], bf, tag="b")
                            cc = ropet.tile([128, 512], bf, tag="c")
                            nc.vector.tensor_mul(a[:], src_ap, cosx[:, o0:o0 + 512])
                            nc.vector.tensor_mul(bb[:], rot[:], sinx[:, o0:o0 + 512])
                            nc.vector.tensor_add(cc[:], a[:], bb[:])
                            dst = (
                                qfin[:, c, o0:o0 + 512]
                                if c < 4
                                else kfin[:, o0:o0 + 512]
                            )
                            nc.vector.tensor_mul(
                                dst, cc[:], scb[:, (o0 % 1024):(o0 % 1024) + 512]
                            )

            # ====== Phases B+C+D+E interleaved (rope / v / attn / o_proj) ==
            with ExitStack() as dctx:
                wop = dctx.enter_context(tc.tile_pool(name="wop", bufs=1))
                attp = dctx.enter_context(tc.tile_pool(name="attnT", bufs=34))
                rnp = dctx.enter_context(tc.tile_pool(name="rnorm", bufs=2))
                ostage = dctx.enter_context(tc.tile_pool(name="ostage", bufs=2))
                sc_psp = dctx.enter_context(
                    tc.tile_pool(name="sc_ps", bufs=3, space="PSUM")
                )
                ot_psp = dctx.enter_context(
                    tc.tile_pool(name="ot_ps", bufs=2, space="PSUM")
                )
                rs_psp = dctx.enter_context(
                    tc.tile_pool(name="rs_ps", bufs=1, space="PSUM")
                )
                op_psp = dctx.enter_context(
                    tc.tile_pool(name="op_ps", bufs=2, space="PSUM")
                )

                wo_sb = wop.tile([128, GQ, H], bf)
                for h in range(GQ):
                    for hh in range(2):
                        sl = slice(hh * 1024, (hh + 1) * 1024)
                        nc.sync.dma_start(
                            out=wo_sb[:, h, sl],
                            in_=wo_d[h * 128:(h + 1) * 128, sl],
                        )
                # ---- attention + o_proj, chunk-major ----
                # software-pipelined by one head: exp tiles for head h are
                # produced while head h-1's attn@v / rowsum matmuls consume
                for ic in range(4):
                    i0 = ic * 512
                    njt = 4 * ic + 4

                    def produce(h):
                        ats = {}
                        jt_order = list(range(4 * ic, njt)) + list(range(4 * ic))
                        for jt in jt_order:
                            t = jt - 4 * ic  # >=0 on diagonal blocks
                            at = attp.tile([128, 512], bf, tag="at", name=f"at_{ic}_{h}_{jt}")
                            if t < 0:
                                sc = sc_psp.tile(
                                    [128, 512], mybir.dt.float32,
                                    tag="sc", name=f"sc_{ic}_{h}_{jt}",
                                )
                                nc.tensor.matmul(
                                    sc[:],
                                    kfin[:, jt * 128:(jt + 1) * 128],
                                    qfin[:, h, i0:i0 + 512],
                                    start=True,
                                    stop=True,
                                )
                                nc.scalar.activation(
                                    at[:], sc[:], Exp, scale=INV_SQRT_HD
                                )
                            else:
                                w = 512 - t * 128
                                sc = sc_psp.tile(
                                    [128, 512], mybir.dt.float32,
                                    tag="sc", name=f"sc_{ic}_{h}_{jt}",
                                )
                                nc.tensor.matmul(
                                    sc[:, :w],
                                    kfin[:, jt * 128:(jt + 1) * 128],
                                    qfin[:, h, i0 + t * 128:i0 + 512],
                                    start=True,
                                    stop=True,
                                )
                                if t > 0:
                                    nc.gpsimd.memset(at[:, :t * 128], 0.0)
                                nc.scalar.activation(
                                    at[:, t * 128:], sc[:, :w], Exp,
                                    scale=INV_SQRT_HD,
                                )
                                nc.gpsimd.tensor_mul(
                                    at[:, t * 128:],
                                    at[:, t * 128:],
                                    masks[:, 0, :w],
                                )
                            ats[jt] = at
                        return ats

                    def consume(h, ats):
                        ot = ot_psp.tile(
                            [128, 512], mybir.dt.float32, tag="ot",
                            name=f"ot_{ic}_{h}",
                        )
                        rs = rs_psp.tile(
                            [128, 512], mybir.dt.float32, tag="rs",
                            name=f"rs_{ic}_{h}",
                        )
                        for jt in range(njt):
                            t = max(jt - 4 * ic, 0) * 128
                            nc.tensor.matmul(
                                ot[:, t:],
                                v3[:, jt, :],
                                ats[jt][:, t:],
                                start=(jt == 0),
                                stop=(jt == njt - 1),
                                skip_group_check=True,
                            )
                        for jt in range(njt):
                            t = max(jt - 4 * ic, 0) * 128
                            nc.tensor.matmul(
                                rs[:, t:],
                                onesm[:],
                                ats[jt][:, t:],
                                start=(jt == 0),
                                stop=(jt == njt - 1),
                                skip_group_check=True,
                            )
                        lnr = rnp.tile([128, 512], mybir.dt.float32, tag="lnr")
                        rr = rnp.tile([128, 512], mybir.dt.float32, tag="rr")
                        nc.scalar.activation(lnr[:], rs[:], Ln)
                        nc.scalar.activation(rr[:], lnr[:], Exp, scale=-1.0)
                        nc.vector.tensor_mul(otsb[:, h, i0:i0 + 512], ot[:], rr[:])

                    def oproj_m(m):
                        ob = ostage.tile([128, H], bf, tag="ob", name=f"ob{m}")
                        for nn in range(4):
                            op = op_psp.tile(
                                [128, 512], mybir.dt.float32, tag="op",
                                name=f"op{m}_{nn}",
                            )
                            for h in range(GQ):
                                nc.tensor.matmul(
                                    op[:],
                                    otsb[:, h, m * 128:(m + 1) * 128],
                                    wo_sb[:, h, nn * 512:(nn + 1) * 512],
                                    start=(h == 0),
                                    stop=(h == GQ - 1),
                                )
                            nc.vector.tensor_copy(
                                ob[:, nn * 512:(nn + 1) * 512], op[:]
                            )
                        for nn in range(4):
                            nc.sync.dma_start(
                                out=out_d[
                                    m * 128:(m + 1) * 128,
                                    nn * 512:(nn + 1) * 512,
                                ],
                                in_=ob[:, nn * 512:(nn + 1) * 512],
                            )

                    prev = None
                    for h in range(GQ):
                        ats = produce(h)
                        if ic > 0:
                            oproj_m((ic - 1) * 4 + h)
                        if prev is not None:
                            consume(*prev)
                        prev = (h, ats)
                    consume(*prev)
                    if ic == 3:
                        for mt in range(4):
                            oproj_m(12 + mt)

    nc.compile()
    return nc


def _get_program():
    global _PROGRAM
    if _PROGRAM is None:
        _PROGRAM = _build_program()
    return _PROGRAM


def _host_consts():
    # rot matrix: out[d', s] = sum_d R[d, d'] t[d, s] = rot(t)[d', s]
    R = np.zeros((128, 128), dtype=np.float32)
    for dp in range(64):
        R[dp + 64, dp] = -1.0
    for dp in range(64, 128):
        R[dp - 64, dp] = 1.0
    ones = np.ones((128, 128), dtype=np.float32)
    # mask[t][p, f] = 1 where key j=(t*128+p) <= query i=f  (within 512 chunk)
    p = np.arange(128)[:, None]
    f = np.arange(512)[None, :]
    mask = np.stack([(t * 128 + p <= f) for t in range(4)]).astype(np.float32)
    return (
        R.astype(BF16),
        ones.astype(BF16),
        np.ascontiguousarray(mask.astype(BF16)),
    )


def kernel(x, sin, cos, Wq, Wk, Wv, Wo, q_norm_w, k_norm_w):
    from concourse.bass_utils import run_bass_kernel_spmd

    nc = _get_program()

    qw = np.asarray(q_norm_w, dtype=np.float32)
    kw = np.asarray(k_norm_w, dtype=np.float32)
    qw_s = np.roll(qw, -64)
    kw_s = np.roll(kw, -64)
    cosT = np.ascontiguousarray(np.asarray(cos, np.float32).T)  # [128, S]
    sinT = np.ascontiguousarray(np.asarray(sin, np.float32).T)
    cosq = (cosT * qw[:, None]).astype(BF16)
    sinq = (sinT * qw_s[:, None]).astype(BF16)
    cosk = (cosT * kw[:, None]).astype(BF16)
    sink = (sinT * kw_s[:, None]).astype(BF16)
    rmat, ones, mask = _host_consts()

    x = np.asarray(x, np.float32)
    xts = [
        np.ascontiguousarray(x[b].T).astype(BF16) for b in range(B)
    ]
    Wq = np.asarray(Wq, np.float32)
    Wk = np.asarray(Wk, np.float32)
    Wv = np.asarray(Wv, np.float32)
    Wo = np.asarray(Wo, np.float32)

    in_maps = []
    for core in range(8):
        b, g = divmod(core, 4)
        in_maps.append(
            {
                "xt": xts[b],
                "wq": np.ascontiguousarray(Wq[:, g * 512:(g + 1) * 512]).astype(BF16),
                "wk": np.ascontiguousarray(Wk[:, g * 128:(g + 1) * 128]).astype(BF16),
                "wv": np.ascontiguousarray(Wv[:, g * 128:(g + 1) * 128]).astype(BF16),
                "wo": np.ascontiguousarray(Wo[g * 512:(g + 1) * 512, :]).astype(BF16),
                "cosq": cosq,
                "sinq": sinq,
                "cosk": cosk,
                "sink": sink,
                "rmat": rmat,
                "ones": ones,
                "mask": mask,
            }
        )

    trace = os.environ.get("KERNEL_TRACE", "0") == "1"
    if trace:
        _inject_ntff_hook()
    res = run_bass_kernel_spmd(nc, in_maps, list(range(8)), trace=trace)
    if trace and res.exec_time_ns is not None:
        print(f"HW exec time: {res.exec_time_ns} ns", file=sys.stderr)
        kernel.last_exec_time_ns = res.exec_time_ns

    out = np.zeros((B, S, H), dtype=np.float32)
    for core in range(8):
        b = core // 4
        out[b] += np.asarray(res.results[core]["out"], dtype=np.float32)
    return out


kernel.last_exec_time_ns = None


def _inject_ntff_hook():
    """Recreate antenv.axon_hooks (absent in this image) so
    run_bass_kernel_spmd(trace=True) can capture NTFF profiles."""
    import types
    import contextlib
    import ctypes

    if "antenv.axon_hooks" in sys.modules:
        return
    so_path = "/opt/axon/libaxon_pjrt.so"
    try:
        lib = ctypes.CDLL(so_path)
        lib.axon_start_nrt_profile.argtypes = [
            ctypes.POINTER(ctypes.c_int64),
            ctypes.c_size_t,
        ]
        lib.axon_start_nrt_profile.restype = ctypes.c_int64
        lib.axon_stop_nrt_profile.argtypes = [ctypes.c_char_p]
        lib.axon_stop_nrt_profile.restype = ctypes.c_int64
    except (OSError, AttributeError):
        return

    @contextlib.contextmanager
    def _hook(output_dir, device_ids):
        import jax

        jax.devices()
        if device_ids:
            ids = (ctypes.c_int64 * len(device_ids))(*device_ids)
            rc = lib.axon_start_nrt_profile(ids, len(device_ids))
        else:
            rc = lib.axon_start_nrt_profile(None, 0)
        if rc != 0:
            raise RuntimeError(f"axon_start_nrt_profile rc={rc}")
        try:
            yield
        finally:
            n = lib.axon_stop_nrt_profile(str(output_dir).encode())
            print(f"profile: {n} file(s) -> {output_dir}", file=sys.stderr)

    mod = types.ModuleType("antenv.axon_hooks")
    mod.get_axon_ntff_profile_hook = lambda: _hook
    sys.modules["antenv.axon_hooks"] = mod



# revision 8
# speedup vs baseline: 1.1314x; 1.1314x over previous
"""Self-contained Trainium2 Bass kernel for nn_Attention_59253368816224.

GQA attention block: per-head RMSNorm on Q/K, RoPE, causal softmax
attention, o_proj.  B=2, S=2048, H=2048, 16 Q heads / 4 KV heads,
head_dim=128.

Sharding: 8 cores = 2 batches x 4 KV groups.  Core c -> (b=c//4, g=c%4)
owns 4 Q heads + 1 KV head.  o_proj is row-parallel: each core emits a
partial [S, H] output contracted over its 512 hidden dims; the host sums
the 4 partials per batch in fp32.

Device pipeline (all matmuls bf16 with fp32 PSUM accumulation):
  A) W-stationary QKV projection from host-pre-transposed xT, producing
     transposed qT/kT/vT [d, s]; per-column sum-of-squares via selector
     matmuls -> RMSNorm scales rsqrt(mean+eps) = exp(-0.5*ln(.)) on ACT.
  B) RoPE in the transposed domain: rot(q) via a constant 128x128
     permutation matmul; combine with w-folded cosT/sinT tables; apply
     the norm scale broadcast across partitions via a DMA row-replicate.
  C) v transposed back to natural [s, d] with a DMA transpose.
  D) Flash-style causal attention on transposed scores [j, i]:
     exp on ACT (scale 1/sqrt(128)), triangle masks on GpSimd, attn@v and
     an all-ones rowsum matmul accumulate in PSUM; normalize via
     exp(-ln(rowsum)) during the PSUM->SBUF move.
  E) o_proj from oT tiles against Wo rows, partial output to DRAM bf16.
"""

import os
import sys
import numpy as np
import ml_dtypes

BF16 = ml_dtypes.bfloat16

B = 2
S = 2048
H = 2048
NQH = 16          # total q heads
NKV = 4           # total kv heads
HD = 128          # head dim
GQ = 4            # q heads per core (per kv group)
KT = H // 128     # 16 k-tiles over hidden
ST = S // 128     # 16 s-tiles
RMS_EPS = 1.1920928955078125e-07
INV_SQRT_HD = 1.0 / float(np.sqrt(HD))

_PROGRAM = None


def _build_program():
    import concourse.bacc as bacc
    import concourse.tile as tile
    from concourse import mybir
    from contextlib import ExitStack

    bf = mybir.dt.bfloat16
    f32 = mybir.dt.float32

    nc = bacc.Bacc("TRN2", target_bir_lowering=False, debug=False, num_devices=8)

    # ---- DRAM I/O (per-core values supplied via in_maps) ----
    xt_d = nc.dram_tensor("xt", (H, S), bf, kind="ExternalInput")
    wq_d = nc.dram_tensor("wq", (H, GQ * HD), bf, kind="ExternalInput")
    wk_d = nc.dram_tensor("wk", (H, HD), bf, kind="ExternalInput")
    wv_d = nc.dram_tensor("wv", (H, HD), bf, kind="ExternalInput")
    wo_d = nc.dram_tensor("wo", (GQ * HD, H), bf, kind="ExternalInput")
    cosq_d = nc.dram_tensor("cosq", (HD, S), bf, kind="ExternalInput")
    sinq_d = nc.dram_tensor("sinq", (HD, S), bf, kind="ExternalInput")
    cosk_d = nc.dram_tensor("cosk", (HD, S), bf, kind="ExternalInput")
    sink_d = nc.dram_tensor("sink", (HD, S), bf, kind="ExternalInput")
    rmat_d = nc.dram_tensor("rmat", (128, 128), bf, kind="ExternalInput")
    ones_d = nc.dram_tensor("ones", (128, 128), bf, kind="ExternalInput")
    mask_d = nc.dram_tensor("mask", (4, 128, 512), bf, kind="ExternalInput")
    out_d = nc.dram_tensor("out", (S, H), bf, kind="ExternalOutput")
    # internal scratch for the scale-row broadcast and the v transpose
    vt_scratch = nc.dram_tensor("vt_scratch", (HD, S), bf)

    Exp = mybir.ActivationFunctionType.Exp
    Sqrt = mybir.ActivationFunctionType.Sqrt

    with tile.TileContext(nc) as tc:
        with ExitStack() as ctx:
            consts = ctx.enter_context(tc.tile_pool(name="consts", bufs=1))
            persist = ctx.enter_context(tc.tile_pool(name="persist", bufs=1))

            # ---- persistent intermediates ----
            qkvbf = persist.tile([128, 5, S], bf)      # raw transposed q(4)/k
            vt = persist.tile([128, S], bf)            # raw transposed v
            qfin = persist.tile([128, GQ, S], bf)      # roped+normed qT
            kfin = persist.tile([128, S], bf)          # roped+normed kT
            v3 = persist.tile([128, ST, HD], bf)       # v natural [jt][j][d]
            otsb = persist.tile([128, GQ, S], bf)      # oT per head

            # ---- constant tiles ----
            cosq = consts.tile([128, S], bf)
            sinq = consts.tile([128, S], bf)
            cosk = consts.tile([128, S], bf)
            sink = consts.tile([128, S], bf)
            rmat = consts.tile([128, 128], bf)
            onesm = consts.tile([128, 128], bf)
            masks = consts.tile([128, 4, 512], bf)
            eps128 = consts.tile([128, 1], mybir.dt.float32)
            nc.vector.memset(eps128[:], RMS_EPS)

            # ============ Phase A: QKV projection (+ per-chunk scales) ====
            CHUNK_ORDER = [4, 5, 0, 1, 2, 3]
            with tc.tile_pool(name="proj_in", bufs=1) as proj_in, \
                 tc.tile_pool(name="sqp", bufs=2) as sqp, \
                 tc.tile_pool(name="scb", bufs=6) as scbp, \
                 tc.tile_pool(name="sqrtp", bufs=2) as sqrtp, \
                 tc.tile_pool(name="ropet", bufs=2) as ropet, \
                 tc.tile_pool(name="qkv_ps", bufs=2, space="PSUM") as qkv_ps, \
                 tc.tile_pool(name="ss_ps", bufs=2, space="PSUM") as ss_ps, \
                 tc.tile_pool(name="rot_ps", bufs=2, space="PSUM") as rot_psp:
                xts = [proj_in.tile([128, S], bf, tag=f"xt{k}", name=f"xt{k}") for k in range(KT)]
                wqs = [proj_in.tile([128, GQ * HD], bf, tag=f"wq{k}", name=f"wqs{k}") for k in range(KT)]
                wks = [proj_in.tile([128, HD], bf, tag=f"wk{k}", name=f"wks{k}") for k in range(KT)]
                wvs = [proj_in.tile([128, HD], bf, tag=f"wv{k}", name=f"wvs{k}") for k in range(KT)]
                # k-major, split issue: early k-tiles land first
                for k in range(KT):
                    for q2 in range(2):
                        nc.sync.dma_start(
                            out=xts[k][:, q2 * 1024:(q2 + 1) * 1024],
                            in_=xt_d[k * 128:(k + 1) * 128, q2 * 1024:(q2 + 1) * 1024],
                        )
                    nc.sync.dma_start(out=wks[k][:], in_=wk_d[k * 128:(k + 1) * 128, :])
                for k in range(KT):
                    nc.sync.dma_start(out=wvs[k][:], in_=wv_d[k * 128:(k + 1) * 128, :])
                for k in range(KT):
                    nc.sync.dma_start(out=wqs[k][:], in_=wq_d[k * 128:(k + 1) * 128, :])
                for hh in range(2):
                    sl = slice(hh * 1024, (hh + 1) * 1024)
                    nc.sync.dma_start(out=cosk[:, sl], in_=cosk_d[:, sl])
                    nc.sync.dma_start(out=sink[:, sl], in_=sink_d[:, sl])
                nc.sync.dma_start(out=rmat[:], in_=rmat_d[:])
                for hh in range(2):
                    sl = slice(hh * 1024, (hh + 1) * 1024)
                    nc.sync.dma_start(out=cosq[:, sl], in_=cosq_d[:, sl])
                    nc.sync.dma_start(out=sinq[:, sl], in_=sinq_d[:, sl])
                nc.sync.dma_start(out=onesm[:], in_=ones_d[:])
                for tt in range(4):
                    nc.sync.dma_start(
                        out=masks[:, tt, :], in_=mask_d[tt, :, :]
                    )

                scbs = {}
                for c in CHUNK_ORDER:
                    for half in range(2):
                        h0 = half * 1024
                        ps = qkv_ps.tile([128, 1024], mybir.dt.float32)
                        for k in range(KT):
                            if c < 4:
                                lhsT = wqs[k][:, c * 128:(c + 1) * 128]
                            elif c == 4:
                                lhsT = wks[k][:]
                            else:
                                lhsT = wvs[k][:]
                            for nn in range(2):
                                nc.tensor.matmul(
                                    ps[:, nn * 512:(nn + 1) * 512],
                                    lhsT,
                                    xts[k][:, h0 + nn * 512:h0 + (nn + 1) * 512],
                                    start=(k == 0),
                                    stop=(k == KT - 1),
                                )
                        dst = (
                            qkvbf[:, c, h0:h0 + 1024]
                            if c < 5
                            else vt[:, h0:h0 + 1024]
                        )
                        if (c + half) % 2 == 0:
                            nc.scalar.copy(dst, ps[:])
                        else:
                            nc.vector.tensor_copy(dst, ps[:])
                        if c < 5:
                            sq = sqp.tile([128, 1024], bf)
                            nc.gpsimd.tensor_mul(sq[:], dst, dst)
                            # scale = rsqrt(mean+eps) = 1/sqrt(mean+eps):
                            # Sqrt on ACT (single table set), recip on DVE
                            for nn in range(2):
                                sst = ss_ps.tile(
                                    [128, 512], mybir.dt.float32,
                                    tag="sst", name=f"sst_{c}_{half}_{nn}",
                                )
                                nc.tensor.matmul(
                                    sst[:],
                                    onesm[:],
                                    sq[:, nn * 512:(nn + 1) * 512],
                                    start=True,
                                    stop=True,
                                )
                                scq = sqrtp.tile(
                                    [128, 512], mybir.dt.float32, tag="scq"
                                )
                                nc.scalar.activation(
                                    scq[:], sst[:], Sqrt,
                                    bias=eps128[:], scale=1.0 / HD,
                                )
                                scb = scbp.tile([128, 512], mybir.dt.float32)
                                scbs[(c, half * 2 + nn)] = scb
                                nc.vector.reciprocal_approx_fast(scb[:], scq[:])
                        elif half == 1:
                            nc.sync.dma_start(out=vt_scratch[:], in_=vt[:])
                            nc.sync.dma_start_transpose(out=v3[:], in_=vt_scratch[:])
                    if c < 5:
                        cosx = cosq if c < 4 else cosk
                        sinx = sinq if c < 4 else sink
                        for q4 in range(4):
                            o0 = q4 * 512
                            scb = scbs[(c, q4)]
                            src_ap = qkvbf[:, c, o0:o0 + 512]
                            rot = rot_psp.tile([128, 512], mybir.dt.float32)
                            nc.tensor.matmul(
                                rot[:], rmat[:], src_ap, start=True, stop=True
                            )
                            a = ropet.tile([128, 512], bf, tag="a")
                            bb = ropet.tile([128, 512], bf, tag="b")
                            cc = ropet.tile([128, 512], bf, tag="c")
                            nc.vector.tensor_mul(a[:], src_ap, cosx[:, o0:o0 + 512])
                            nc.vector.tensor_mul(bb[:], rot[:], sinx[:, o0:o0 + 512])
                            nc.vector.tensor_add(cc[:], a[:], bb[:])
                            dst = (
                                qfin[:, c, o0:o0 + 512]
                                if c < 4
                                else kfin[:, o0:o0 + 512]
                            )
                            nc.vector.tensor_mul(dst, cc[:], scb[:])

            # ====== Phases B+C+D+E interleaved (rope / v / attn / o_proj) ==
            with ExitStack() as dctx:
                wop = dctx.enter_context(tc.tile_pool(name="wop", bufs=1))
                attp = dctx.enter_context(tc.tile_pool(name="attnT", bufs=34))
                rnp = dctx.enter_context(tc.tile_pool(name="rnorm", bufs=2))
                ostage = dctx.enter_context(tc.tile_pool(name="ostage", bufs=2))
                sc_psp = dctx.enter_context(
                    tc.tile_pool(name="sc_ps", bufs=3, space="PSUM")
                )
                ot_psp = dctx.enter_context(
                    tc.tile_pool(name="ot_ps", bufs=2, space="PSUM")
                )
                rs_psp = dctx.enter_context(
                    tc.tile_pool(name="rs_ps", bufs=1, space="PSUM")
                )
                op_psp = dctx.enter_context(
                    tc.tile_pool(name="op_ps", bufs=2, space="PSUM")
                )

                wo_sb = wop.tile([128, GQ, H], bf)
                for h in range(GQ):
                    for hh in range(2):
                        sl = slice(hh * 1024, (hh + 1) * 1024)
                        nc.sync.dma_start(
                            out=wo_sb[:, h, sl],
                            in_=wo_d[h * 128:(h + 1) * 128, sl],
                        )
                # ---- attention + o_proj, chunk-major ----
                # software-pipelined by one head: exp tiles for head h are
                # produced while head h-1's attn@v / rowsum matmuls consume
                for ic in range(4):
                    i0 = ic * 512
                    njt = 4 * ic + 4

                    def produce(h):
                        ats = {}
                        jt_order = list(range(4 * ic, njt)) + list(range(4 * ic))
                        for jt in jt_order:
                            t = jt - 4 * ic  # >=0 on diagonal blocks
                            at = attp.tile([128, 512], bf, tag="at", name=f"at_{ic}_{h}_{jt}")
                            if t < 0:
                                sc = sc_psp.tile(
                                    [128, 512], mybir.dt.float32,
                                    tag="sc", name=f"sc_{ic}_{h}_{jt}",
                                )
                                nc.tensor.matmul(
                                    sc[:],
                                    kfin[:, jt * 128:(jt + 1) * 128],
                                    qfin[:, h, i0:i0 + 512],
                                    start=True,
                                    stop=True,
                                )
                                nc.scalar.activation(
                                    at[:], sc[:], Exp, scale=INV_SQRT_HD
                                )
                            else:
                                w = 512 - t * 128
                                sc = sc_psp.tile(
                                    [128, 512], mybir.dt.float32,
                                    tag="sc", name=f"sc_{ic}_{h}_{jt}",
                                )
                                nc.tensor.matmul(
                                    sc[:, :w],
                                    kfin[:, jt * 128:(jt + 1) * 128],
                                    qfin[:, h, i0 + t * 128:i0 + 512],
                                    start=True,
                                    stop=True,
                                )
                                nc.scalar.activation(
                                    at[:, t * 128:], sc[:, :w], Exp,
                                    scale=INV_SQRT_HD,
                                )
                                # only the leading 128 cols need the triangle
                                # mask; beyond that every key in this tile is
                                # visible
                                nc.vector.tensor_mul(
                                    at[:, t * 128:t * 128 + 128],
                                    at[:, t * 128:t * 128 + 128],
                                    masks[:, 0, :128],
                                )
                            ats[jt] = at
                        return ats

                    def consume(h, ats):
                        ot = ot_psp.tile(
                            [128, 512], mybir.dt.float32, tag="ot",
                            name=f"ot_{ic}_{h}",
                        )
                        rs = rs_psp.tile(
                            [128, 512], mybir.dt.float32, tag="rs",
                            name=f"rs_{ic}_{h}",
                        )
                        for jt in range(njt):
                            t = max(jt - 4 * ic, 0) * 128
                            nc.tensor.matmul(
                                ot[:, t:],
                                v3[:, jt, :],
                                ats[jt][:, t:],
                                start=(jt == 0),
                                stop=(jt == njt - 1),
                                skip_group_check=True,
                            )
                        for jt in range(njt):
                            t = max(jt - 4 * ic, 0) * 128
                            nc.tensor.matmul(
                                rs[:, t:],
                                onesm[:],
                                ats[jt][:, t:],
                                start=(jt == 0),
                                stop=(jt == njt - 1),
                                skip_group_check=True,
                            )
                        rr = rnp.tile([128, 512], mybir.dt.float32, tag="rr")
                        nc.vector.reciprocal_approx_fast(rr[:], rs[:])
                        nc.vector.tensor_mul(otsb[:, h, i0:i0 + 512], ot[:], rr[:])

                    def oproj_m(m):
                        ob = ostage.tile([128, H], bf, tag="ob", name=f"ob{m}")
                        for nn in range(4):
                            op = op_psp.tile(
                                [128, 512], mybir.dt.float32, tag="op",
                                name=f"op{m}_{nn}",
                            )
                            for h in range(GQ):
                                nc.tensor.matmul(
                                    op[:],
                                    otsb[:, h, m * 128:(m + 1) * 128],
                                    wo_sb[:, h, nn * 512:(nn + 1) * 512],
                                    start=(h == 0),
                                    stop=(h == GQ - 1),
                                )
                            nc.vector.tensor_copy(
                                ob[:, nn * 512:(nn + 1) * 512], op[:]
                            )
                        for nn in range(4):
                            nc.sync.dma_start(
                                out=out_d[
                                    m * 128:(m + 1) * 128,
                                    nn * 512:(nn + 1) * 512,
                                ],
                                in_=ob[:, nn * 512:(nn + 1) * 512],
                            )

                    prev = None
                    for h in range(GQ):
                        ats = produce(h)
                        if ic > 0:
                            oproj_m((ic - 1) * 4 + h)
                        if prev is not None:
                            consume(*prev)
                        prev = (h, ats)
                    consume(*prev)
                    if ic == 3:
                        for mt in range(4):
                            oproj_m(12 + mt)

    nc.compile()
    return nc


def _get_program():
    global _PROGRAM
    if _PROGRAM is None:
        _PROGRAM = _build_program()
    return _PROGRAM


def _host_consts():
    # rot matrix: out[d', s] = sum_d R[d, d'] t[d, s] = rot(t)[d', s]
    R = np.zeros((128, 128), dtype=np.float32)
    for dp in range(64):
        R[dp + 64, dp] = -1.0
    for dp in range(64, 128):
        R[dp - 64, dp] = 1.0
    ones = np.ones((128, 128), dtype=np.float32)
    # mask[t][p, f] = 1 where key j=(t*128+p) <= query i=f  (within 512 chunk)
    p = np.arange(128)[:, None]
    f = np.arange(512)[None, :]
    mask = np.stack([(t * 128 + p <= f) for t in range(4)]).astype(np.float32)
    return (
        R.astype(BF16),
        ones.astype(BF16),
        np.ascontiguousarray(mask.astype(BF16)),
    )


def kernel(x, sin, cos, Wq, Wk, Wv, Wo, q_norm_w, k_norm_w):
    from concourse.bass_utils import run_bass_kernel_spmd

    nc = _get_program()

    qw = np.asarray(q_norm_w, dtype=np.float32)
    kw = np.asarray(k_norm_w, dtype=np.float32)
    qw_s = np.roll(qw, -64)
    kw_s = np.roll(kw, -64)
    cosT = np.ascontiguousarray(np.asarray(cos, np.float32).T)  # [128, S]
    sinT = np.ascontiguousarray(np.asarray(sin, np.float32).T)
    cosq = (cosT * qw[:, None]).astype(BF16)
    sinq = (sinT * qw_s[:, None]).astype(BF16)
    cosk = (cosT * kw[:, None]).astype(BF16)
    sink = (sinT * kw_s[:, None]).astype(BF16)
    rmat, ones, mask = _host_consts()

    x = np.asarray(x, np.float32)
    xts = [
        np.ascontiguousarray(x[b].T).astype(BF16) for b in range(B)
    ]
    Wq = np.asarray(Wq, np.float32)
    Wk = np.asarray(Wk, np.float32)
    Wv = np.asarray(Wv, np.float32)
    Wo = np.asarray(Wo, np.float32)

    in_maps = []
    for core in range(8):
        b, g = divmod(core, 4)
        in_maps.append(
            {
                "xt": xts[b],
                "wq": np.ascontiguousarray(Wq[:, g * 512:(g + 1) * 512]).astype(BF16),
                "wk": np.ascontiguousarray(Wk[:, g * 128:(g + 1) * 128]).astype(BF16),
                "wv": np.ascontiguousarray(Wv[:, g * 128:(g + 1) * 128]).astype(BF16),
                "wo": np.ascontiguousarray(Wo[g * 512:(g + 1) * 512, :]).astype(BF16),
                "cosq": cosq,
                "sinq": sinq,
                "cosk": cosk,
                "sink": sink,
                "rmat": rmat,
                "ones": ones,
                "mask": mask,
            }
        )

    trace = os.environ.get("KERNEL_TRACE", "0") == "1"
    if trace:
        _inject_ntff_hook()
    res = run_bass_kernel_spmd(nc, in_maps, list(range(8)), trace=trace)
    if trace and res.exec_time_ns is not None:
        print(f"HW exec time: {res.exec_time_ns} ns", file=sys.stderr)
        kernel.last_exec_time_ns = res.exec_time_ns

    out = np.zeros((B, S, H), dtype=np.float32)
    for core in range(8):
        b = core // 4
        out[b] += np.asarray(res.results[core]["out"], dtype=np.float32)
    return out


kernel.last_exec_time_ns = None


def _inject_ntff_hook():
    """Recreate antenv.axon_hooks (absent in this image) so
    run_bass_kernel_spmd(trace=True) can capture NTFF profiles."""
    import types
    import contextlib
    import ctypes

    if "antenv.axon_hooks" in sys.modules:
        return
    so_path = "/opt/axon/libaxon_pjrt.so"
    try:
        lib = ctypes.CDLL(so_path)
        lib.axon_start_nrt_profile.argtypes = [
            ctypes.POINTER(ctypes.c_int64),
            ctypes.c_size_t,
        ]
        lib.axon_start_nrt_profile.restype = ctypes.c_int64
        lib.axon_stop_nrt_profile.argtypes = [ctypes.c_char_p]
        lib.axon_stop_nrt_profile.restype = ctypes.c_int64
    except (OSError, AttributeError):
        return

    @contextlib.contextmanager
    def _hook(output_dir, device_ids):
        import jax

        jax.devices()
        if device_ids:
            ids = (ctypes.c_int64 * len(device_ids))(*device_ids)
            rc = lib.axon_start_nrt_profile(ids, len(device_ids))
        else:
            rc = lib.axon_start_nrt_profile(None, 0)
        if rc != 0:
            raise RuntimeError(f"axon_start_nrt_profile rc={rc}")
        try:
            yield
        finally:
            n = lib.axon_stop_nrt_profile(str(output_dir).encode())
            print(f"profile: {n} file(s) -> {output_dir}", file=sys.stderr)

    mod = types.ModuleType("antenv.axon_hooks")
    mod.get_axon_ntff_profile_hook = lambda: _hook
    sys.modules["antenv.axon_hooks"] = mod



# revision 23
# speedup vs baseline: 1.2186x; 1.0771x over previous
"""Self-contained Trainium2 Bass kernel for nn_Attention_59253368816224.

GQA attention block: per-head RMSNorm on Q/K, RoPE, causal softmax
attention, o_proj.  B=2, S=2048, H=2048, 16 Q heads / 4 KV heads,
head_dim=128.

Sharding: 8 cores = 2 batches x 4 KV groups.  Core c -> (b=c//4, g=c%4)
owns 4 Q heads + 1 KV head.  o_proj is row-parallel: each core emits a
partial [S, H] output contracted over its 512 hidden dims; the host sums
the 4 partials per batch in fp32.

Device pipeline (all matmuls bf16 with fp32 PSUM accumulation):
  A) W-stationary QKV projection from host-pre-transposed xT, producing
     transposed qT/kT/vT [d, s]; per-column sum-of-squares via selector
     matmuls -> RMSNorm scales rsqrt(mean+eps) = exp(-0.5*ln(.)) on ACT.
  B) RoPE in the transposed domain: rot(q) via a constant 128x128
     permutation matmul; combine with w-folded cosT/sinT tables; apply
     the norm scale broadcast across partitions via a DMA row-replicate.
  C) v transposed back to natural [s, d] with a DMA transpose.
  D) Flash-style causal attention on transposed scores [j, i]:
     exp on ACT (scale 1/sqrt(128)), triangle masks on GpSimd, attn@v and
     an all-ones rowsum matmul accumulate in PSUM; normalize via
     exp(-ln(rowsum)) during the PSUM->SBUF move.
  E) o_proj from oT tiles against Wo rows, partial output to DRAM bf16.
"""

import os
import sys
import numpy as np
import ml_dtypes

BF16 = ml_dtypes.bfloat16

B = 2
S = 2048
H = 2048
NQH = 16          # total q heads
NKV = 4           # total kv heads
HD = 128          # head dim
GQ = 4            # q heads per core (per kv group)
KT = H // 128     # 16 k-tiles over hidden
ST = S // 128     # 16 s-tiles
RMS_EPS = 1.1920928955078125e-07
INV_SQRT_HD = 1.0 / float(np.sqrt(HD))

_PROGRAM = None


def _build_program():
    import concourse.bacc as bacc
    import concourse.tile as tile
    from concourse import mybir
    from contextlib import ExitStack

    bf = mybir.dt.bfloat16
    f32 = mybir.dt.float32

    nc = bacc.Bacc("TRN2", target_bir_lowering=False, debug=False, num_devices=8)

    # ---- DRAM I/O (per-core values supplied via in_maps) ----
    xt_d = nc.dram_tensor("xt", (KT, 128, S), bf, kind="ExternalInput")
    wq_d = nc.dram_tensor("wq", (H, GQ * HD), bf, kind="ExternalInput")
    wk_d = nc.dram_tensor("wk", (H, HD), bf, kind="ExternalInput")
    wv_d = nc.dram_tensor("wv", (H, HD), bf, kind="ExternalInput")
    wo_d = nc.dram_tensor("wo", (GQ * HD, H), bf, kind="ExternalInput")
    cosq_d = nc.dram_tensor("cosq", (HD, S), bf, kind="ExternalInput")
    sinq_d = nc.dram_tensor("sinq", (HD, S), bf, kind="ExternalInput")
    cosk_d = nc.dram_tensor("cosk", (HD, S), bf, kind="ExternalInput")
    sink_d = nc.dram_tensor("sink", (HD, S), bf, kind="ExternalInput")
    rmat_d = nc.dram_tensor("rmat", (128, 128), bf, kind="ExternalInput")
    ones_d = nc.dram_tensor("ones", (128, 128), bf, kind="ExternalInput")
    mask_d = nc.dram_tensor("mask", (128, 128), bf, kind="ExternalInput")
    out_d = nc.dram_tensor("out", (S, H), bf, kind="ExternalOutput")
    # internal scratch for the scale-row broadcast and the v transpose
    vt_scratch = nc.dram_tensor("vt_scratch", (HD, S), bf)

    Exp = mybir.ActivationFunctionType.Exp
    Sqrt = mybir.ActivationFunctionType.Sqrt

    with tile.TileContext(nc) as tc:
        with ExitStack() as ctx:
            consts = ctx.enter_context(tc.tile_pool(name="consts", bufs=1))
            persist = ctx.enter_context(tc.tile_pool(name="persist", bufs=1))

            # ---- persistent intermediates ----
            qkvbf = persist.tile([128, 5, S], bf)      # raw transposed q(4)/k
            vt = persist.tile([128, S], bf)            # raw transposed v
            qfin = persist.tile([128, GQ, S], bf)      # roped+normed qT
            kfin = persist.tile([128, S], bf)          # roped+normed kT
            v3 = persist.tile([128, ST, HD], bf)       # v natural [jt][j][d]
            otsb = persist.tile([128, GQ, S], bf)      # oT per head
            wo_sb = persist.tile([128, GQ, H], bf)     # o_proj weights

            # ---- constant tiles ----
            cosq = consts.tile([128, S], bf)
            sinq = consts.tile([128, S], bf)
            cosk = consts.tile([128, S], bf)
            sink = consts.tile([128, S], bf)
            rmat = consts.tile([128, 128], bf)
            onesm = consts.tile([128, 128], bf)
            masks = consts.tile([128, 128], bf)
            eps128 = consts.tile([128, 1], mybir.dt.float32)
            nc.vector.memset(eps128[:], RMS_EPS)

            # ============ Phase A: QKV projection (+ per-chunk scales) ====
            CHUNK_ORDER = [4, 5, 0, 1, 2, 3]
            with tc.tile_pool(name="proj_in", bufs=1) as proj_in, \
                 tc.tile_pool(name="sqp", bufs=2) as sqp, \
                 tc.tile_pool(name="scb", bufs=6) as scbp, \
                 tc.tile_pool(name="ropet", bufs=2) as ropet, \
                 tc.tile_pool(name="qkv_ps", bufs=2, space="PSUM") as qkv_ps, \
                 tc.tile_pool(name="ss_ps", bufs=2, space="PSUM") as ss_ps, \
                 tc.tile_pool(name="rot_ps", bufs=2, space="PSUM") as rot_psp:
                xts = [proj_in.tile([128, S], bf, tag=f"xt{k}", name=f"xt{k}") for k in range(KT)]
                wqs = [proj_in.tile([128, GQ * HD], bf, tag=f"wq{k}", name=f"wqs{k}") for k in range(KT)]
                wks = [proj_in.tile([128, HD], bf, tag=f"wk{k}", name=f"wks{k}") for k in range(KT)]
                wvs = [proj_in.tile([128, HD], bf, tag=f"wv{k}", name=f"wvs{k}") for k in range(KT)]
                # k-major, split issue: early k-tiles land first
                for k in range(KT):
                    nc.sync.dma_start(out=xts[k][:], in_=xt_d[k, :, :])
                    nc.sync.dma_start(out=wks[k][:], in_=wk_d[k * 128:(k + 1) * 128, :])
                for k in range(KT):
                    nc.sync.dma_start(out=wvs[k][:], in_=wv_d[k * 128:(k + 1) * 128, :])
                for k in range(KT):
                    nc.sync.dma_start(out=wqs[k][:], in_=wq_d[k * 128:(k + 1) * 128, :])
                for hh in range(2):
                    sl = slice(hh * 1024, (hh + 1) * 1024)
                    nc.sync.dma_start(out=cosk[:, sl], in_=cosk_d[:, sl])
                    nc.sync.dma_start(out=sink[:, sl], in_=sink_d[:, sl])
                nc.sync.dma_start(out=rmat[:], in_=rmat_d[:])
                for hh in range(2):
                    sl = slice(hh * 1024, (hh + 1) * 1024)
                    nc.sync.dma_start(out=cosq[:, sl], in_=cosq_d[:, sl])
                    nc.sync.dma_start(out=sinq[:, sl], in_=sinq_d[:, sl])
                nc.sync.dma_start(out=onesm[:], in_=ones_d[:])
                nc.sync.dma_start(out=masks[:], in_=mask_d[:])
                # o_proj weights stream during the post-input DMA lull so
                # phase D doesn't wait on them
                for h in range(GQ):
                    for hh in range(2):
                        sl = slice(hh * 1024, (hh + 1) * 1024)
                        nc.sync.dma_start(
                            out=wo_sb[:, h, sl],
                            in_=wo_d[h * 128:(h + 1) * 128, sl],
                        )

                scbs = {}
                for c in CHUNK_ORDER:
                    for half in range(2):
                        h0 = half * 1024
                        ps = qkv_ps.tile([128, 1024], mybir.dt.float32)
                        for k in range(KT):
                            if c < 4:
                                lhsT = wqs[k][:, c * 128:(c + 1) * 128]
                            elif c == 4:
                                lhsT = wks[k][:]
                            else:
                                lhsT = wvs[k][:]
                            for nn in range(2):
                                nc.tensor.matmul(
                                    ps[:, nn * 512:(nn + 1) * 512],
                                    lhsT,
                                    xts[k][:, h0 + nn * 512:h0 + (nn + 1) * 512],
                                    start=(k == 0),
                                    stop=(k == KT - 1),
                                )
                        dst = (
                            qkvbf[:, c, h0:h0 + 1024]
                            if c < 5
                            else vt[:, h0:h0 + 1024]
                        )
                        if (c + half) % 2 == 0:
                            nc.scalar.copy(dst, ps[:])
                        else:
                            nc.vector.tensor_copy(dst, ps[:])
                        if c < 5:
                            sq = sqp.tile([128, 1024], bf)
                            nc.gpsimd.tensor_mul(sq[:], dst, dst)
                            # scale = rsqrt(mean+eps) = 1/sqrt(mean+eps):
                            # Sqrt on ACT (single table set), recip on DVE
                            for nn in range(2):
                                sst = ss_ps.tile(
                                    [128, 512], mybir.dt.float32,
                                    tag="sst", name=f"sst_{c}_{half}_{nn}",
                                )
                                nc.tensor.matmul(
                                    sst[:],
                                    onesm[:],
                                    sq[:, nn * 512:(nn + 1) * 512],
                                    start=True,
                                    stop=True,
                                )
                                scb = scbp.tile([128, 512], mybir.dt.float32)
                                scbs[(c, half * 2 + nn)] = scb
                                nc.scalar.activation(
                                    scb[:], sst[:], Sqrt,
                                    bias=eps128[:], scale=1.0 / HD,
                                )
                                nc.vector.reciprocal_approx_fast(scb[:], scb[:])
                        elif half == 1:
                            nc.sync.dma_start(out=vt_scratch[:], in_=vt[:])
                            nc.sync.dma_start_transpose(out=v3[:], in_=vt_scratch[:])
                    if c < 5:
                        cosx = cosq if c < 4 else cosk
                        sinx = sinq if c < 4 else sink
                        for q4 in range(4):
                            o0 = q4 * 512
                            scb = scbs[(c, q4)]
                            src_ap = qkvbf[:, c, o0:o0 + 512]
                            rot = rot_psp.tile([128, 512], mybir.dt.float32)
                            nc.tensor.matmul(
                                rot[:], rmat[:], src_ap, start=True, stop=True
                            )
                            a = ropet.tile([128, 512], bf, tag="a")
                            bb = ropet.tile([128, 512], bf, tag="b")
                            cc = ropet.tile([128, 512], bf, tag="c")
                            nc.vector.tensor_mul(a[:], src_ap, cosx[:, o0:o0 + 512])
                            nc.vector.tensor_mul(bb[:], rot[:], sinx[:, o0:o0 + 512])
                            nc.vector.tensor_add(cc[:], a[:], bb[:])
                            dst = (
                                qfin[:, c, o0:o0 + 512]
                                if c < 4
                                else kfin[:, o0:o0 + 512]
                            )
                            nc.vector.tensor_mul(dst, cc[:], scb[:])

            # ====== Phases B+C+D+E interleaved (rope / v / attn / o_proj) ==
            with ExitStack() as dctx:
                attp = dctx.enter_context(tc.tile_pool(name="attnT", bufs=34))
                rnp = dctx.enter_context(tc.tile_pool(name="rnorm", bufs=2))
                ostage = dctx.enter_context(tc.tile_pool(name="ostage", bufs=2))
                sc_psp = dctx.enter_context(
                    tc.tile_pool(name="sc_ps", bufs=3, space="PSUM")
                )
                ot_psp = dctx.enter_context(
                    tc.tile_pool(name="ot_ps", bufs=2, space="PSUM")
                )
                rs_psp = dctx.enter_context(
                    tc.tile_pool(name="rs_ps", bufs=1, space="PSUM")
                )
                op_psp = dctx.enter_context(
                    tc.tile_pool(name="op_ps", bufs=2, space="PSUM")
                )

                # ---- attention + o_proj, chunk-major ----
                # software-pipelined by one head: exp tiles for head h are
                # produced while head h-1's attn@v / rowsum matmuls consume
                for ic in range(4):
                    i0 = ic * 512
                    njt = 4 * ic + 4

                    def produce(h):
                        ats = {}
                        gsums = []
                        jt_order = list(range(4 * ic, njt)) + list(range(4 * ic))
                        for jt in jt_order:
                            t = jt - 4 * ic  # >=0 on diagonal blocks
                            at = attp.tile([128, 512], bf, tag="at", name=f"at_{ic}_{h}_{jt}")
                            if t < 0:
                                sc = sc_psp.tile(
                                    [128, 512], mybir.dt.float32,
                                    tag="sc", name=f"sc_{ic}_{h}_{jt}",
                                )
                                nc.tensor.matmul(
                                    sc[:],
                                    kfin[:, jt * 128:(jt + 1) * 128],
                                    qfin[:, h, i0:i0 + 512],
                                    start=True,
                                    stop=True,
                                )
                                nc.scalar.activation(
                                    at[:], sc[:], Exp, scale=INV_SQRT_HD
                                )
                            else:
                                w = 512 - t * 128
                                sc = sc_psp.tile(
                                    [128, 512], mybir.dt.float32,
                                    tag="sc", name=f"sc_{ic}_{h}_{jt}",
                                )
                                nc.tensor.matmul(
                                    sc[:, :w],
                                    kfin[:, jt * 128:(jt + 1) * 128],
                                    qfin[:, h, i0 + t * 128:i0 + 512],
                                    start=True,
                                    stop=True,
                                )
                                nc.scalar.activation(
                                    at[:, t * 128:], sc[:, :w], Exp,
                                    scale=INV_SQRT_HD,
                                )
                                # only the leading 128 cols need the triangle
                                # mask; beyond that every key in this tile is
                                # visible
                                nc.vector.tensor_mul(
                                    at[:, t * 128:t * 128 + 128],
                                    at[:, t * 128:t * 128 + 128],
                                    masks[:],
                                )
                            ats[jt] = at
                        # pre-sum each full off-diagonal group of 4 key tiles
                        # on DVE so the rowsum needs 1 matmul per group
                        for g in range(ic):
                            u0 = attp.tile([128, 512], bf, tag="gsu", bufs=4,
                                           name=f"gsu0_{ic}_{h}_{g}")
                            u1 = attp.tile([128, 512], bf, tag="gsu", bufs=4,
                                           name=f"gsu1_{ic}_{h}_{g}")
                            gs = attp.tile([128, 512], bf, tag="gs", bufs=8,
                                           name=f"gs_{ic}_{h}_{g}")
                            nc.vector.tensor_add(
                                u0[:], ats[4 * g][:], ats[4 * g + 1][:]
                            )
                            nc.vector.tensor_add(
                                u1[:], ats[4 * g + 2][:], ats[4 * g + 3][:]
                            )
                            nc.vector.tensor_add(gs[:], u0[:], u1[:])
                            gsums.append(gs)
                        return ats, gsums

                    def consume(h, ats, gsums):
                        ot = ot_psp.tile(
                            [128, 512], mybir.dt.float32, tag="ot",
                            name=f"ot_{ic}_{h}",
                        )
                        rs = rs_psp.tile(
                            [128, 512], mybir.dt.float32, tag="rs",
                            name=f"rs_{ic}_{h}",
                        )
                        for jt in range(njt):
                            t = max(jt - 4 * ic, 0) * 128
                            nc.tensor.matmul(
                                ot[:, t:],
                                v3[:, jt, :],
                                ats[jt][:, t:],
                                start=(jt == 0),
                                stop=(jt == njt - 1),
                                skip_group_check=True,
                            )
                        for g in range(ic):
                            nc.tensor.matmul(
                                rs[:],
                                onesm[:],
                                gsums[g][:],
                                start=(g == 0),
                                stop=False,
                                skip_group_check=True,
                            )
                        for dt_ in range(4):
                            jt = 4 * ic + dt_
                            t = dt_ * 128
                            nc.tensor.matmul(
                                rs[:, t:],
                                onesm[:],
                                ats[jt][:, t:],
                                start=(ic == 0 and dt_ == 0),
                                stop=(dt_ == 3),
                                skip_group_check=True,
                            )
                        rr = rnp.tile([128, 512], mybir.dt.float32, tag="rr")
                        nc.vector.reciprocal_approx_fast(rr[:], rs[:])
                        nc.vector.tensor_mul(otsb[:, h, i0:i0 + 512], ot[:], rr[:])

                    def oproj_m(m):
                        ob = ostage.tile([128, H], bf, tag="ob", name=f"ob{m}")
                        for nn in range(4):
                            op = op_psp.tile(
                                [128, 512], mybir.dt.float32, tag="op",
                                name=f"op{m}_{nn}",
                            )
                            for h in range(GQ):
                                nc.tensor.matmul(
                                    op[:],
                                    otsb[:, h, m * 128:(m + 1) * 128],
                                    wo_sb[:, h, nn * 512:(nn + 1) * 512],
                                    start=(h == 0),
                                    stop=(h == GQ - 1),
                                )
                            if nn % 2 == 0:
                                nc.scalar.copy(
                                    ob[:, nn * 512:(nn + 1) * 512], op[:]
                                )
                            else:
                                nc.vector.tensor_copy(
                                    ob[:, nn * 512:(nn + 1) * 512], op[:]
                                )
                        for nn in range(4):
                            nc.sync.dma_start(
                                out=out_d[
                                    m * 128:(m + 1) * 128,
                                    nn * 512:(nn + 1) * 512,
                                ],
                                in_=ob[:, nn * 512:(nn + 1) * 512],
                            )

                    prev = None
                    for h in range(GQ):
                        ats, gsums = produce(h)
                        if ic > 0:
                            oproj_m((ic - 1) * 4 + h)
                        if prev is not None:
                            consume(*prev)
                        prev = (h, ats, gsums)
                    consume(*prev)
                    if ic == 3:
                        for mt in range(4):
                            oproj_m(12 + mt)

    nc.compile()
    return nc


def _get_program():
    global _PROGRAM
    if _PROGRAM is None:
        _PROGRAM = _build_program()
    return _PROGRAM


def _host_consts():
    # rot matrix: out[d', s] = sum_d R[d, d'] t[d, s] = rot(t)[d', s]
    R = np.zeros((128, 128), dtype=np.float32)
    for dp in range(64):
        R[dp + 64, dp] = -1.0
    for dp in range(64, 128):
        R[dp - 64, dp] = 1.0
    ones = np.ones((128, 128), dtype=np.float32)
    # mask[p, f] = 1 where key offset p <= query offset f (diagonal block)
    p = np.arange(128)[:, None]
    f = np.arange(128)[None, :]
    mask = (p <= f).astype(np.float32)
    return (
        R.astype(BF16),
        ones.astype(BF16),
        np.ascontiguousarray(mask.astype(BF16)),
    )


def kernel(x, sin, cos, Wq, Wk, Wv, Wo, q_norm_w, k_norm_w):
    from concourse.bass_utils import run_bass_kernel_spmd

    nc = _get_program()

    qw = np.asarray(q_norm_w, dtype=np.float32)
    kw = np.asarray(k_norm_w, dtype=np.float32)
    qw_s = np.roll(qw, -64)
    kw_s = np.roll(kw, -64)
    cosT = np.ascontiguousarray(np.asarray(cos, np.float32).T)  # [128, S]
    sinT = np.ascontiguousarray(np.asarray(sin, np.float32).T)
    cosq = (cosT * qw[:, None]).astype(BF16)
    sinq = (sinT * qw_s[:, None]).astype(BF16)
    cosk = (cosT * kw[:, None]).astype(BF16)
    sink = (sinT * kw_s[:, None]).astype(BF16)
    rmat, ones, mask = _host_consts()

    x = np.asarray(x, np.float32)
    # pack xT k-tile-contiguous: [KT, 128, S] so each k-tile is one DMA
    # with 4KB-contiguous partition lines
    xts = [
        np.ascontiguousarray(
            x[b].T.reshape(KT, 128, S)
        ).astype(BF16)
        for b in range(B)
    ]
    Wq = np.asarray(Wq, np.float32)
    Wk = np.asarray(Wk, np.float32)
    Wv = np.asarray(Wv, np.float32)
    Wo = np.asarray(Wo, np.float32)

    in_maps = []
    for core in range(8):
        b, g = divmod(core, 4)
        in_maps.append(
            {
                "xt": xts[b],
                "wq": np.ascontiguousarray(Wq[:, g * 512:(g + 1) * 512]).astype(BF16),
                "wk": np.ascontiguousarray(Wk[:, g * 128:(g + 1) * 128]).astype(BF16),
                "wv": np.ascontiguousarray(Wv[:, g * 128:(g + 1) * 128]).astype(BF16),
                "wo": np.ascontiguousarray(Wo[g * 512:(g + 1) * 512, :]).astype(BF16),
                "cosq": cosq,
                "sinq": sinq,
                "cosk": cosk,
                "sink": sink,
                "rmat": rmat,
                "ones": ones,
                "mask": mask,
            }
        )

    trace = os.environ.get("KERNEL_TRACE", "0") == "1"
    if trace:
        _inject_ntff_hook()
    res = run_bass_kernel_spmd(nc, in_maps, list(range(8)), trace=trace)
    if trace and res.exec_time_ns is not None:
        print(f"HW exec time: {res.exec_time_ns} ns", file=sys.stderr)
        kernel.last_exec_time_ns = res.exec_time_ns

    out = np.zeros((B, S, H), dtype=np.float32)
    for core in range(8):
        b = core // 4
        out[b] += np.asarray(res.results[core]["out"], dtype=np.float32)
    return out


kernel.last_exec_time_ns = None


def _inject_ntff_hook():
    """Recreate antenv.axon_hooks (absent in this image) so
    run_bass_kernel_spmd(trace=True) can capture NTFF profiles."""
    import types
    import contextlib
    import ctypes

    if "antenv.axon_hooks" in sys.modules:
        return
    so_path = "/opt/axon/libaxon_pjrt.so"
    try:
        lib = ctypes.CDLL(so_path)
        lib.axon_start_nrt_profile.argtypes = [
            ctypes.POINTER(ctypes.c_int64),
            ctypes.c_size_t,
        ]
        lib.axon_start_nrt_profile.restype = ctypes.c_int64
        lib.axon_stop_nrt_profile.argtypes = [ctypes.c_char_p]
        lib.axon_stop_nrt_profile.restype = ctypes.c_int64
    except (OSError, AttributeError):
        return

    @contextlib.contextmanager
    def _hook(output_dir, device_ids):
        import jax

        jax.devices()
        if device_ids:
            ids = (ctypes.c_int64 * len(device_ids))(*device_ids)
            rc = lib.axon_start_nrt_profile(ids, len(device_ids))
        else:
            rc = lib.axon_start_nrt_profile(None, 0)
        if rc != 0:
            raise RuntimeError(f"axon_start_nrt_profile rc={rc}")
        try:
            yield
        finally:
            n = lib.axon_stop_nrt_profile(str(output_dir).encode())
            print(f"profile: {n} file(s) -> {output_dir}", file=sys.stderr)

    mod = types.ModuleType("antenv.axon_hooks")
    mod.get_axon_ntff_profile_hook = lambda: _hook
    sys.modules["antenv.axon_hooks"] = mod

